# revision 1
# baseline (speedup 1.0000x reference)
"""Trainium2 Bass kernel for nn_MixedActivation.

Column i of x uses activation (i % 6): 0,1,2 -> square; 3,4,5 -> PReLU with
prelu_a[0..2]. Data-parallel over rows across 8 NeuronCores (125000 rows
each); the three PReLU scalars are baked into each core's program as
immediates.

The kernel is DMA-bound: all traffic rides the 16-SDMA fleet per core
(~435 GB/s each direction, full duplex), so bytes moved set the floor.
With the 2e-2 relative-error budget the tensor travels as bf16 both ways
(rel err ~1.1e-2, dominated by the squared columns): the host downcasts x
during sharding, the device streams 12 MB in + 12 MB out per core, and the
host upcasts the result to f32. The same structure with f32 I/O runs
~111 us; this runs ~69 us.

Layout: per-core shard [125000, 48] bf16 processed as 10 tiles where
partition p holds B=100 consecutive rows (48*B contiguous elements in
DRAM). The mod-6 column pattern becomes a period-6 pattern along the free
dim, covered by strided-AP instructions computing in place:
  - squares (phases 0,1,2): DVE tensor_tensor mult, inner run of 3
  - PReLU  (phases 3,4,5): ACT Prelu; the three alphas are equal in the
    reference (0.25), which fuses all three phases into ONE run-3
    instruction per tile. Fallbacks for unequal alphas keep correctness
    (offloading one phase to DVE as max(a*x, x) when 0 <= a <= 1).
In-DMAs issue on SP's HWDGE ring, out-DMAs on ACT's HWDGE ring, bitcast
to uint32 (identical bytes, 4-byte costing). Per-slot semaphores keep
DMA completion counts exact; with NB = NTILES slots the whole shard is
SBUF-resident and there are no write-after-read reloads.
"""

import numpy as np

import concourse.bass as bass
import concourse.mybir as mybir
from concourse.bass_utils import run_bass_kernel_spmd

N_CORES = 8
ROWS = 1_000_000
COLS = 48
SHARD_ROWS = ROWS // N_CORES  # 125000

B = 100                 # rows per partition per tile
P = 128                 # partitions
TILE_ROWS = P * B       # 12800
NB = 10                 # buffer slots; 10 = whole shard resident, no WAR
F = COLS * B            # 4800 elements per partition


def _build(prelu_a, replicas=1, B=B, NB=NB):
    """Build the per-core BIR program (bf16 in-place compute, all HWDGE).

    SP issues bf16 in-DMAs on its HWDGE ring; DVE squares its strided view
    in place; ACT applies PReLU in place (single fused run-3 instruction
    when the three alphas are equal, as in the reference) and issues the
    bf16 out-DMA on its HWDGE ring. Per-slot sems keep DMA completion
    counts exact; with NB >= NTILES there are no WAR reloads.

    replicas>1 unrolls the whole pipeline K times over the same data -
    used only for timing (K-replica differencing isolates HW exec time
    from host/dispatch overhead).
    """
    TILE_ROWS = P * B
    N_FULL = SHARD_ROWS // TILE_ROWS
    TAIL_ROWS = SHARD_ROWS - N_FULL * TILE_ROWS
    assert TAIL_ROWS % B == 0
    TAIL_P = TAIL_ROWS // B
    NTILES = N_FULL + (1 if TAIL_ROWS else 0)
    F = COLS * B
    a0, a1, a2 = (float(v) for v in prelu_a)
    # prelu dispatch: 'fused' = one run-3 Prelu on ACT (equal alphas, the
    # reference case); 'split' = phases 3,4 on ACT + phase 5 on DVE as
    # max(a2*x, x) (valid for 0 <= a2 <= 1); 'generic' = 3 Prelus on ACT
    if a0 == a1 == a2:
        mode = "fused"
    elif 0.0 <= a2 <= 1.0:
        mode = "split"
    else:
        mode = "generic"
    _orig_preamble = bass.BassEngine.preamble
    bass.BassEngine.preamble = lambda self: None
    try:
        nc = bass.Bass("TRN2", target_bir_lowering=False)
    finally:
        bass.BassEngine.preamble = _orig_preamble
    x_ext = nc.declare_dram_parameter(
        "x", [SHARD_ROWS, COLS], mybir.dt.bfloat16, isOutput=False
    )
    y_ext = nc.declare_dram_parameter(
        "y", [SHARD_ROWS, COLS], mybir.dt.bfloat16, isOutput=True
    )

    # DRAM tile views: [n, p, b*c] with contiguous per-partition chunks
    x_full = x_ext[0 : N_FULL * TILE_ROWS, :].rearrange(
        "(n p b) c -> n p (b c)", n=N_FULL, p=P, b=B
    )
    y_full = y_ext[0 : N_FULL * TILE_ROWS, :].rearrange(
        "(n p b) c -> n p (b c)", n=N_FULL, p=P, b=B
    )
    if TAIL_ROWS:
        x_tail = x_ext[N_FULL * TILE_ROWS :, :].rearrange(
            "(p b) c -> p (b c)", p=TAIL_P, b=B
        )
        y_tail = y_ext[N_FULL * TILE_ROWS :, :].rearrange(
            "(p b) c -> p (b c)", p=TAIL_P, b=B
        )

    def dram_in(i):
        return x_full[i] if i < N_FULL else x_tail

    def dram_out(i):
        return y_full[i] if i < N_FULL else y_tail

    def pdim(i):
        return P if i < N_FULL else TAIL_P

    from contextlib import ExitStack

    with ExitStack() as stack:
        tin = stack.enter_context(
            nc.sbuf_tensor([P, NB * F], mybir.dt.bfloat16)
        )
        in_sems = [
            stack.enter_context(nc.semaphore(f"in_sem{b}")) for b in range(NB)
        ]
        out_sems = [
            stack.enter_context(nc.semaphore(f"out_sem{b}")) for b in range(NB)
        ]
        sq_sem = stack.enter_context(nc.semaphore("sq_sem"))
        pr_sem = stack.enter_context(nc.semaphore("pr_sem"))
        block = stack.enter_context(nc.Block())

        NT = NTILES * replicas

        def dti(t):  # schedule index -> dram tile index
            return t % NTILES

        def buf(t):
            return tin[: pdim(dti(t)), (t % NB) * F : (t % NB + 1) * F]

        def n_loads(t):  # value of in_sems[t % NB] after load of tile t
            return 16 * (t // NB + 1)

        @block.sync
        def _(sync):
            for t in range(NT):
                if t >= NB:
                    # WAR: reload slot only after out-DMA t-NB fully read it
                    sync.wait_ge(out_sems[t % NB], n_loads(t - NB))
                sync.dma_start(
                    out=buf(t).bitcast(mybir.dt.uint32),
                    in_=dram_in(dti(t)).bitcast(mybir.dt.uint32),
                ).then_inc(in_sems[t % NB], 16)
            for b in range(min(NB, NT)):
                last_t = NT - 1 - (NT - 1 - b) % NB  # last schedule slot on b
                sync.wait_ge(out_sems[b], n_loads(last_t))

        @block.vector
        def _(vector):
            for t in range(NT):
                vector.wait_ge(in_sems[t % NB], n_loads(t))
                v = buf(t).rearrange("p (b g s) -> p b g s", b=B, g=8, s=6)
                vector.tensor_tensor(
                    out=v[:, :, :, 0:3],
                    in0=v[:, :, :, 0:3],
                    in1=v[:, :, :, 0:3],
                    op=mybir.AluOpType.mult,
                )
                if mode == "split":
                    # prelu(x) = max(a*x, x) for 0 <= a <= 1
                    vector.scalar_tensor_tensor(
                        out=v[:, :, :, 5:6],
                        in0=v[:, :, :, 5:6],
                        scalar=a2,
                        in1=v[:, :, :, 5:6],
                        op0=mybir.AluOpType.mult,
                        op1=mybir.AluOpType.max,
                    )
                vector.drain().then_inc(sq_sem, 1)

        @block.scalar
        def _(scalar):
            for t in range(NT):
                i = dti(t)
                scalar.wait_ge(in_sems[t % NB], n_loads(t))
                v = buf(t).rearrange("p (b g s) -> p b g s", b=B, g=8, s=6)
                if mode == "fused":
                    scalar.activation(
                        out=v[:, :, :, 3:6],
                        in_=v[:, :, :, 3:6],
                        func=mybir.ActivationFunctionType.Prelu,
                        alpha=a0,
                    )
                else:
                    nk = 2 if mode == "split" else 3
                    for k, a in list(enumerate((a0, a1, a2)))[:nk]:
                        scalar.activation(
                            out=v[:, :, :, 3 + k : 4 + k],
                            in_=v[:, :, :, 3 + k : 4 + k],
                            func=mybir.ActivationFunctionType.Prelu,
                            alpha=a,
                        )
                # drain flushes ACT's SBUF writes before the out-DMA reads
                scalar.drain().then_inc(pr_sem, 1)
                scalar.wait_ge(sq_sem, t + 1)
                scalar.dma_start(
                    out=dram_out(i).bitcast(mybir.dt.uint32),
                    in_=buf(t).bitcast(mybir.dt.uint32),
                ).then_inc(out_sems[t % NB], 16)

    return nc


def kernel(x: np.ndarray, prelu_a: np.ndarray, trace: bool = False):
    import ml_dtypes

    nc = _build(prelu_a)
    xb = np.ascontiguousarray(x, dtype=np.float32).astype(ml_dtypes.bfloat16)
    in_maps = [
        {"x": xb[c * SHARD_ROWS : (c + 1) * SHARD_ROWS]} for c in range(N_CORES)
    ]
    res = run_bass_kernel_spmd(nc, in_maps, list(range(N_CORES)), trace=trace)
    out = np.concatenate(
        [np.asarray(res.results[c]["y"]).astype(np.float32) for c in range(N_CORES)],
        axis=0,
    )
    if trace:
        return out, res
    return out

